# revision 37
# baseline (speedup 1.0000x reference)
"""Trainium2 Bass kernel for nn_CopiedSetEncoder (set encoder with recurrent
attention). Self-contained: shards batch across 8 NeuronCores with a balanced
contiguous token packing, builds a length-specialized SPMD Tile kernel in
bf16, runs it, and reassembles the output.

Structure per core (16 sequences packed into T tokens):
  phase 1  MLP over 512-token tiles -> embA [E-major] and embB [token-major]
           (embB via PE transposes of embA, not extra matmuls)
  phase 2  5 recurrent-attention iterations; softmax uses unnormalized
           bf16 exp weights, per-group DVE reductions for the sums, and
           column-tiled (4x concurrent) attended matmuls; LSTM activations
           are computed from Exp only (no activation-table swaps).
"""
import numpy as np
import ml_dtypes

import concourse.bass as bass
import concourse.mybir as mybir
import concourse.tile as tile
from concourse.bass_utils import run_bass_kernel_spmd

B, F_, D_IN = 128, 1024, 128
H1, H2, E, H = 512, 512, 256, 256
N_SHUFFLE = 5
NCORES = 8
BLOC = B // NCORES  # 16 sequences per core
NEG = -1e30

f32 = mybir.dt.float32
bf16 = mybir.dt.bfloat16
BF = ml_dtypes.bfloat16
AF = mybir.ActivationFunctionType
OP = mybir.AluOpType


def _split_multi_waits(nc):
    """HW allows at most one sync wait per instruction; hoist extras into
    standalone InstEventSemaphore carriers on the same engine."""
    cnt = 0
    for bb in nc.main_func.blocks:
        insts = bb.instructions  # live list
        i = 0
        while i < len(insts):
            ins = insts[i]
            si = ins.sync_info
            if si is not None and si.on_wait and len(si.on_wait) > 1:
                waits = list(si.on_wait)
                carriers = []
                for w in waits[:-1]:
                    cnt += 1
                    ev = mybir.InstEventSemaphore(name=f"wsplit-{cnt}")
                    ev.engine = ins.engine
                    ev.sync_info = mybir.SyncInfo(on_wait=[w], on_update=[])
                    carriers.append(ev)
                ins.sync_info = mybir.SyncInfo(
                    on_wait=[waits[-1]], on_update=list(si.on_update)
                )
                for j, ev in enumerate(carriers):
                    insts.insert(i + j, ev)
                    nc.register_instruction(ev, overwrite=True)
                i += len(carriers)
            i += 1
    return cnt


def _view(t_ap, offset_elems, dims):
    """Build a strided free-dim view of a tile AP. dims = [[stride, size], ...]
    for the free dims; partition dim copied from the tile."""
    return bass.AP(
        tensor=t_ap.tensor,
        offset=t_ap.offset + offset_elems,
        ap=[list(t_ap.ap[0])] + [list(d) for d in dims],
    )


def _build_module(T):
    C = T // 128          # 128-token chunks
    NT = T // 512         # MLP tiles
    NG = (C + 7) // 8     # softmax groups of up to 8 chunks
    NQ = C // 4           # attended quads (C is a multiple of 4)

    nc = bass.Bass()

    # ---- inputs ----
    xT_e = nc.declare_dram_parameter("xT", [128, T], bf16, isOutput=False)
    w1_e = nc.declare_dram_parameter("w1", [128, H1], bf16, isOutput=False)
    w2a_e = nc.declare_dram_parameter("w2a", [128, 2, H2], mybir.dt.float8e4,
                                      isOutput=False)
    w2b_e = nc.declare_dram_parameter("w2b", [128, 2, H2], bf16, isOutput=False)
    w3_e = nc.declare_dram_parameter("w3", [128, 4, E], bf16, isOutput=False)
    wih_e = nc.declare_dram_parameter("wih", [128, 2, 4 * H], bf16, isOutput=False)
    whh_e = nc.declare_dram_parameter("whh", [128, 2, 4 * H], bf16, isOutput=False)
    b1_e = nc.declare_dram_parameter("b1", [128, 4], f32, isOutput=False)
    b2_e = nc.declare_dram_parameter("b2", [128, 4], f32, isOutput=False)
    bg_e = nc.declare_dram_parameter("bg", [1, 8 * 128], bf16, isOutput=False)
    ones16_e = nc.declare_dram_parameter("ones16", [1, BLOC], bf16, isOutput=False)
    mask_e = nc.declare_dram_parameter("mask", [128, C, BLOC], f32, isOutput=False)
    w0T_e = nc.declare_dram_parameter("w0T", [128, C, BLOC], bf16, isOutput=False)
    sel_e = nc.declare_dram_parameter("sel", [128, BLOC], bf16, isOutput=False)
    ident_e = nc.declare_dram_parameter("ident", [128, 128], bf16, isOutput=False)
    onesc_e = nc.declare_dram_parameter("onesc", [128, 1], f32, isOutput=False)
    # constants for the activation-table toggle dummies: sigma(37)=1.0 exactly,
    # exp(-80)~0; their outputs feed scale/bias APs to pin scheduling order
    c37_e = nc.declare_dram_parameter("c37", [128, 1], f32, isOutput=False)
    cm80_e = nc.declare_dram_parameter("cm80", [128, 1], f32, isOutput=False)
    att_o = nc.declare_dram_parameter("att", [BLOC, E], f32, isOutput=True)
    qt_o = nc.declare_dram_parameter("qt", [128, 2, BLOC], f32, isOutput=True)

    with tile.TileContext(nc) as tc:
        with tc.tile_pool(name="big", bufs=1) as big, \
             tc.tile_pool(name="wp", bufs=1) as wp:
            xT = big.tile([128, T], bf16)
            embA = big.tile([128, 2, T], bf16)
            embB = big.tile([128, C, E], bf16)
            w1T = big.tile([128, C, BLOC], bf16)
            Spart = big.tile([128, NG, BLOC], f32)
            w1 = wp.tile([128, H1], bf16)
            w2a = wp.tile([128, 2, H2], mybir.dt.float8e4)
            w2b = wp.tile([128, 2, H2], bf16)
            w3 = wp.tile([128, 4, E], bf16)
            wih = wp.tile([128, 2, 4 * H], bf16)
            whh = wp.tile([128, 2, 4 * H], bf16)
            b1 = wp.tile([128, 4], f32)
            b2 = wp.tile([128, 4], f32)
            bg = wp.tile([1, 8 * 128], bf16)
            ones16 = wp.tile([1, BLOC], bf16)
            mask = wp.tile([128, C, BLOC], f32)
            w0T = wp.tile([128, C, BLOC], bf16)
            sel = wp.tile([128, BLOC], bf16)
            ident = wp.tile([128, 128], bf16)
            onesc = wp.tile([128, 1], f32)
            c37 = wp.tile([128, 1], f32)
            cm80 = wp.tile([128, 1], f32)

            # weight DMAs needed by the first MLP tile go first; xT is
            # DMA'd per tile inside the loop; attention-only inputs later.
            for dst, src in [(w1, w1_e), (b1, b1_e), (w2a, w2a_e), (w2b, w2b_e), (b2, b2_e),
                             (w3, w3_e), (ident, ident_e)]:
                nc.sync.dma_start(out=dst[:], in_=src[:])

            # ---- phase 1: MLP over 512-token tiles ----
            with tc.tile_pool(name="mlp", bufs=3) as mp, \
                 tc.tile_pool(name="ps1", bufs=2, space="PSUM") as ps1, \
                 tc.tile_pool(name="ps2", bufs=2, space="PSUM") as ps2, \
                 tc.tile_pool(name="ps3", bufs=2, space="PSUM") as ps3, \
                 tc.tile_pool(name="psE", bufs=2, space="PSUM") as psE:
                for t in range(NT):
                    sl = slice(t * 512, (t + 1) * 512)
                    nc.sync.dma_start(out=xT[:, sl], in_=xT_e[:, sl])
                    if t == 1:
                        # attention-phase inputs, overlapped with compute
                        for dst, src in [(mask, mask_e), (w0T, w0T_e),
                                         (wih, wih_e), (whh, whh_e),
                                         (bg, bg_e), (ones16, ones16_e),
                                         (sel, sel_e), (onesc, onesc_e),
                                         (c37, c37_e), (cm80, cm80_e)]:
                            nc.sync.dma_start(out=dst[:], in_=src[:])
                    h1a = mp.tile([128, 2, 512], mybir.dt.float8e4, tag="h1a")
                    h1b = mp.tile([128, 2, 512], bf16, tag="h1b")
                    for mc in range(4):
                        p = ps1.tile([128, 512], f32, tag="pA")
                        nc.tensor.matmul(
                            p[:], w1[:, mc * 128:(mc + 1) * 128], xT[:, sl],
                            start=True, stop=True,
                        )
                        hdst = (h1a[:, mc, :] if mc < 2
                                else h1b[:, mc - 2, :])
                        if mc % 2 == 0:
                            nc.scalar.activation(
                                out=hdst, in_=p[:], func=AF.Relu,
                                bias=b1[:, mc:mc + 1], scale=1.0,
                            )
                        else:
                            nc.vector.tensor_scalar(
                                out=hdst, in0=p[:],
                                scalar1=b1[:, mc:mc + 1], scalar2=0.0,
                                op0=OP.add, op1=OP.max,
                            )
                    h2t = mp.tile([128, 4, 512], bf16, tag="h2")
                    for mc in range(4):
                        p = ps2.tile([128, 512], f32, tag="pB")
                        nc.tensor.matmul(
                            p[:], w2a[:, :, mc * 128:(mc + 1) * 128],
                            h1a[:, :, :], start=True, stop=False,
                            perf_mode=mybir.MatmulPerfMode.DoubleRow,
                        )
                        for kc in range(2):
                            nc.tensor.matmul(
                                p[:], w2b[:, kc, mc * 128:(mc + 1) * 128],
                                h1b[:, kc, :], start=False, stop=(kc == 1),
                            )
                        # undo the x64 * 8192 fp8 scaling inside the relu
                        if mc % 2 == 0:
                            nc.scalar.activation(
                                out=h2t[:, mc, :], in_=p[:], func=AF.Relu,
                                bias=b2[:, mc:mc + 1], scale=2.0 ** -17,
                            )
                        else:
                            nc.vector.scalar_tensor_tensor(
                                out=h2t[:, mc, :], in0=p[:],
                                scalar=2.0 ** -17,
                                in1=_view(b2[:], mc, [[0, 512]]),
                                op0=OP.mult, op1=OP.add,
                            )
                            nc.vector.tensor_scalar(
                                out=h2t[:, mc, :], in0=h2t[:, mc, :],
                                scalar1=0.0, scalar2=0.0,
                                op0=OP.max, op1=OP.add,
                            )
                    for mc in range(2):
                        p = ps3.tile([128, 512], f32, tag="pC")
                        for kc in range(4):
                            nc.tensor.matmul(
                                p[:], w3[:, kc, mc * 128:(mc + 1) * 128],
                                h2t[:, kc, :], start=(kc == 0), stop=(kc == 3),
                            )
                        nc.scalar.copy(out=embA[:, mc, sl], in_=p[:])
                    # embB (token-major) via PE transposes of embA
                    for s in range(4):
                        for kc in range(2):
                            pt = psE.tile([128, 128], bf16, tag="pT")
                            nc.tensor.transpose(
                                pt[:],
                                embA[:, kc, t * 512 + s * 128:
                                     t * 512 + (s + 1) * 128],
                                ident[:, :],
                            )
                            nc.vector.tensor_copy(
                                embB[:, t * 4 + s, kc * 128:(kc + 1) * 128],
                                pt[:],
                            )

            # ---- phase 2: recurrent attention ----
            with tc.tile_pool(name="att", bufs=1) as ap, \
                 tc.tile_pool(name="attd", bufs=2) as ad, \
                 tc.tile_pool(name="psL", bufs=2, space="PSUM") as psL, \
                 tc.tile_pool(name="psA", bufs=1, space="PSUM") as psA, \
                 tc.tile_pool(name="psG", bufs=1, space="PSUM") as psG, \
                 tc.tile_pool(name="psC", bufs=1, space="PSUM") as psC, \
                 tc.tile_pool(name="psT", bufs=1, space="PSUM") as psT:
                qtT = ap.tile([128, 2, BLOC], bf16)
                qtT32 = ap.tile([128, 2, BLOC], f32)
                ct = ap.tile([128, 2, BLOC], f32)
                att_bf = ap.tile([BLOC, E], bf16)
                att_f = ap.tile([BLOC, E], f32)
                attT = ap.tile([128, 2, BLOC], bf16)
                S_sb = ap.tile([128, BLOC], f32)
                rS = ap.tile([BLOC, 1], f32)
                dsig = ap.tile([128, 1], f32)  # 1.0, pins sigmoid-table load
                dexp = ap.tile([128, 1], f32)  # ~0.0, pins exp-table load
                att_ps = psA.tile([128, E], f32)
                nc.vector.memset(qtT[:], 0.0)
                nc.vector.memset(att_ps[:], 0.0)  # dead rows stay 0 forever

                w1ap = w1T[:]
                spap = Spart[:]

                def emit_att_quad(q, wsrc):
                    for j in range(4):
                        c = 4 * q + j
                        nc.tensor.matmul(
                            att_ps[32 * j:32 * j + BLOC, :],
                            wsrc[:, c, :], embB[:, c, :],
                            start=(q == 0), stop=(q == NQ - 1),
                            tile_position=(0, 32 * j), skip_group_check=True,
                        )

                for it in range(N_SHUFFLE):
                    if it > 0:
                        # logits (chunk-stationary), masked exp per group,
                        # attended quads of group g-1 interleaved
                        for g in range(NG):
                            nch = min(8, C - 8 * g)
                            lgp = psL.tile([128, 8, BLOC], f32, tag="lgp")
                            for ci in range(nch):
                                c = 8 * g + ci
                                for kc in range(2):
                                    nc.tensor.matmul(
                                        lgp[:, ci, :],
                                        embA[:, kc, c * 128:(c + 1) * 128],
                                        qtT[:, kc, :],
                                        start=(kc == 0), stop=(kc == 1),
                                    )
                            lgm = ad.tile([128, 8, BLOC], f32, tag="lgm")
                            nc.vector.tensor_tensor(
                                out=lgm[:, :nch, :], in0=lgp[:, :nch, :],
                                in1=mask[:, 8 * g:8 * g + nch, :], op=OP.add,
                            )
                            nc.scalar.activation(
                                out=w1T[:, 8 * g:8 * g + nch, :],
                                in_=lgm[:, :nch, :], func=AF.Exp,
                                bias=dexp[:, 0:1],
                            )
                            gview = _view(w1ap, 8 * g * BLOC,
                                          [[1, BLOC], [BLOC, nch]])
                            nc.vector.tensor_reduce(
                                out=Spart[:, g, :], in_=gview,
                                axis=mybir.AxisListType.X, op=OP.add,
                            )
                            for qq in range(2 * g - 2, 2 * g):
                                if 0 <= qq < NQ:
                                    emit_att_quad(qq, w1T)
                        # preload the sigmoid table while the PE runs the
                        # remaining attended and gates matmuls; the bias AP
                        # pins this after the last exp in the ACT stream
                        nc.scalar.activation(out=dsig[:], in_=c37[:],
                                             func=AF.Sigmoid,
                                             bias=w1T[:, C - 1, 0:1])
                        for qq in range(2 * NG - 2, NQ):
                            emit_att_quad(qq, w1T)
                        # S and 1/S
                        sview = _view(spap, 0, [[1, BLOC], [BLOC, NG]])
                        nc.vector.tensor_reduce(
                            out=S_sb[:], in_=sview,
                            axis=mybir.AxisListType.X, op=OP.add,
                        )
                        s_ps = psT.tile([BLOC, 1], f32, tag="sps")
                        nc.tensor.matmul(s_ps[:], S_sb[:], onesc[:],
                                         start=True, stop=True)
                        nc.vector.reciprocal(rS[:], s_ps[:])
                    else:
                        nc.scalar.activation(out=dsig[:], in_=c37[:],
                                             func=AF.Sigmoid)
                        for qq in range(NQ):
                            emit_att_quad(qq, w0T)

                    # combine 4 column partials via selector matmul
                    attC = ad.tile([128, E], bf16, tag="attC")
                    nc.vector.tensor_copy(attC[:], att_ps[:])
                    comb = psC.tile([BLOC, E], f32, tag="comb")
                    nc.tensor.matmul(comb[:], sel[:], attC[:],
                                     start=True, stop=True)
                    if it == 0:
                        nc.vector.tensor_copy(att_bf[:], comb[:])
                    else:
                        nc.vector.tensor_scalar(
                            out=att_bf[:], in0=comb[:], scalar1=rS[:],
                            scalar2=0.0, op0=OP.mult, op1=OP.add,
                        )
                        if it == N_SHUFFLE - 1:
                            nc.vector.tensor_scalar(
                                out=att_f[:], in0=comb[:], scalar1=rS[:],
                                scalar2=0.0, op0=OP.mult, op1=OP.add,
                            )
                    for c2 in range(2):
                        pt = psT.tile([128, BLOC], bf16, tag="ptA")
                        nc.tensor.transpose(
                            pt[:], att_bf[:, c2 * 128:(c2 + 1) * 128],
                            ident[:BLOC, :BLOC],
                        )
                        nc.vector.tensor_copy(attT[:, c2, :], pt[:])

                    # LSTM gates; chunk order [i0 i1 f0 f1 o0 o1 g0 g1],
                    # bias folded in as a rank-1 (K=1) matmul per chunk
                    g_ps = psG.tile([128, 8, BLOC], f32, tag="g")
                    for mc in range(8):
                        msl = slice(mc * 128, (mc + 1) * 128)
                        nc.tensor.matmul(
                            g_ps[:, mc, :], wih[:, 0, msl], attT[:, 0, :],
                            start=True, stop=False,
                        )
                        nc.tensor.matmul(
                            g_ps[:, mc, :], wih[:, 1, msl], attT[:, 1, :],
                            start=False, stop=False,
                        )
                        if it > 0:
                            for kc in range(2):
                                nc.tensor.matmul(
                                    g_ps[:, mc, :], whh[:, kc, msl],
                                    qtT[:, kc, :], start=False, stop=False,
                                )
                        nc.tensor.matmul(
                            g_ps[:, mc, :], bg[:, msl], ones16[:],
                            start=False, stop=True,
                        )
                    # sigmoid-only nonlinearities (one activation table):
                    # tanh(x) = 2*sigmoid(2x) - 1, g-gate rows are pre-doubled
                    # on the host. dsig == 1.0 as a scale AP pins these after
                    # the sigmoid-table toggle in the ACT stream.
                    sig6 = ad.tile([128, 6, BLOC], f32, tag="sig6")
                    nc.scalar.activation(out=sig6[:], in_=g_ps[:, 0:6, :],
                                         func=AF.Sigmoid, scale=dsig[:, 0:1])
                    sg2 = ad.tile([128, 2, BLOC], f32, tag="sg2")
                    nc.scalar.activation(out=sg2[:], in_=g_ps[:, 6:8, :],
                                         func=AF.Sigmoid, scale=dsig[:, 0:1])
                    th2 = ad.tile([128, 2, BLOC], f32, tag="th2")
                    nc.vector.tensor_scalar(out=th2[:], in0=sg2[:], scalar1=2.0,
                                            scalar2=-1.0, op0=OP.mult,
                                            op1=OP.add)
                    if it == 0:
                        nc.vector.tensor_tensor(out=ct[:], in0=sig6[:, 0:2, :],
                                                in1=th2[:], op=OP.mult)
                    else:
                        tmp = ad.tile([128, 2, BLOC], f32, tag="tmp")
                        nc.vector.tensor_tensor(out=tmp[:], in0=sig6[:, 0:2, :],
                                                in1=th2[:], op=OP.mult)
                        nc.vector.tensor_tensor(out=ct[:], in0=sig6[:, 2:4, :],
                                                in1=ct[:], op=OP.mult)
                        nc.vector.tensor_tensor(out=ct[:], in0=ct[:],
                                                in1=tmp[:], op=OP.add)
                    sgc = ad.tile([128, 2, BLOC], f32, tag="sgc")
                    nc.scalar.activation(out=sgc[:], in_=ct[:],
                                         func=AF.Sigmoid, scale=2.0)
                    # swap the table back to Exp under the next logits pass;
                    # the bias AP pins this after sgc in the ACT stream
                    nc.scalar.activation(out=dexp[:], in_=cm80[:], func=AF.Exp,
                                         bias=sgc[:, 0, 0:1])
                    thc = ad.tile([128, 2, BLOC], f32, tag="thc")
                    nc.vector.tensor_scalar(out=thc[:], in0=sgc[:], scalar1=2.0,
                                            scalar2=-1.0, op0=OP.mult,
                                            op1=OP.add)
                    if it == N_SHUFFLE - 1:
                        nc.vector.tensor_tensor(out=qtT32[:],
                                                in0=sig6[:, 4:6, :],
                                                in1=thc[:], op=OP.mult)
                    else:
                        nc.vector.tensor_tensor(out=qtT[:],
                                                in0=sig6[:, 4:6, :],
                                                in1=thc[:], op=OP.mult)
                        # keepalive: tiny matmuls tied to mid-chain tensors so
                        # the PE never sees a fully idle HAM window
                        for src in (ct, thc):
                            jp = psT.tile([BLOC, 1], f32, tag="sps")
                            nc.tensor.matmul(jp[:], src[:, 0, :], onesc[:],
                                             start=True, stop=True)

                nc.sync.dma_start(out=att_o[:], in_=att_f[:])
                nc.sync.dma_start(out=qt_o[:], in_=qtT32[:])

    _split_multi_waits(nc)
    return nc


def kernel(state, length, W1, b1, W2, b2, W3, b3, W_ih, W_hh, b_ih, b_hh):
    state = np.asarray(state, dtype=np.float32)
    lengths = np.asarray(length).astype(np.int64)

    # balanced assignment: greedy longest-first onto least-loaded core
    # that still has a free slot (16 per core)
    order = np.argsort(-lengths, kind="stable")
    core_slots = [[] for _ in range(NCORES)]
    core_sum = np.zeros(NCORES, dtype=np.int64)
    for idx in order:
        free = [c for c in range(NCORES) if len(core_slots[c]) < BLOC]
        c = min(free, key=lambda c: core_sum[c])
        core_slots[c].append(int(idx))
        core_sum[c] += lengths[idx]
    T = int(-(-int(core_sum.max()) // 512) * 512)
    C = T // 128

    nc = _build_module(T)

    # shared weights, matmul-ready layouts. L2 runs in fp8 DoubleRow:
    # x is pre-scaled by 64 and W2 by 8192 so both operands sit in the
    # e4m3 normal range; the combined 2^-19 is undone in the L2 relu.
    w1h = np.ascontiguousarray(W1.T).astype(BF)
    w2ah = np.ascontiguousarray(
        (W2[:, 0:256] * 4096.0).T.reshape(2, 128, H2).transpose(1, 0, 2)
    ).astype(ml_dtypes.float8_e4m3)
    w2bh = np.ascontiguousarray(
        (W2[:, 256:512] * 4096.0).T.reshape(2, 128, H2).transpose(1, 0, 2)
    ).astype(BF)
    w3h = np.ascontiguousarray(
        W3.T.reshape(4, 128, E).transpose(1, 0, 2)).astype(BF)
    # gate-chunk reorder [i f g o] -> [i f o g] so sigmoid gates are
    # contiguous; the tanh gate rows are doubled since the device computes
    # tanh(x) as 2*sigmoid(2x)-1
    ridx = np.r_[0:512, 768:1024, 512:768]
    gsc = np.ones(4 * H, dtype=np.float32)
    gsc[768:] = 2.0
    wihh = np.ascontiguousarray(
        (W_ih[ridx] * gsc[:, None]).T.reshape(2, 128, 4 * H)
        .transpose(1, 0, 2)).astype(BF)
    whhh = np.ascontiguousarray(
        (W_hh[ridx] * gsc[:, None]).T.reshape(2, 128, 4 * H)
        .transpose(1, 0, 2)).astype(BF)
    bgv = ((b_ih + b_hh + W_ih @ b3)[ridx] * gsc).astype(np.float32)
    bgh = np.ascontiguousarray(bgv.reshape(1, 8 * 128)).astype(BF)
    ones16 = np.ones((1, BLOC), dtype=BF)
    c37h = np.full((128, 1), 37.0, dtype=np.float32)
    cm80h = np.full((128, 1), -80.0, dtype=np.float32)
    b1h = np.ascontiguousarray((b1 * 32.0).reshape(4, 128).T).astype(np.float32)
    b2h = np.ascontiguousarray(b2.reshape(4, 128).T).astype(np.float32)
    selh = np.zeros((128, BLOC), dtype=BF)
    for k in range(4):
        for j in range(BLOC):
            selh[32 * k + j, j] = 1.0
    identh = np.eye(128, dtype=BF)
    onesh = np.ones((128, 1), dtype=np.float32)

    in_maps = []
    for c in range(NCORES):
        xT = np.zeros((128, T), dtype=BF)
        maskh = np.full((128, C, BLOC), NEG, dtype=np.float32)
        w0T = np.zeros((128, C, BLOC), dtype=BF)
        off = 0
        for j, seq in enumerate(core_slots[c]):
            ln = int(lengths[seq])
            xT[:, off:off + ln] = (state[seq, :ln, :].T * 32.0).astype(BF)
            tt = np.arange(off, off + ln)
            maskh[tt % 128, tt // 128, j] = 0.0
            w0T[tt % 128, tt // 128, j] = BF(1.0 / ln)
            off += ln
        in_maps.append({
            "xT": xT, "w1": w1h, "w2a": w2ah, "w2b": w2bh, "w3": w3h,
            "wih": wihh, "whh": whhh, "b1": b1h, "b2": b2h, "bg": bgh,
            "ones16": ones16, "mask": maskh, "w0T": w0T, "sel": selh,
            "ident": identh, "onesc": onesh, "c37": c37h, "cm80": cm80h,
        })

    res = run_bass_kernel_spmd(nc, in_maps, list(range(NCORES)))

    out = np.zeros((B, E + H), dtype=np.float32)
    b3f = b3.astype(np.float32)
    for c in range(NCORES):
        att = np.asarray(res.results[c]["att"], dtype=np.float32)
        qt = np.asarray(res.results[c]["qt"], dtype=np.float32)  # [128,2,16]
        for j, seq in enumerate(core_slots[c]):
            out[seq, :E] = att[j] + b3f
            out[seq, E:E + 128] = qt[:, 0, j]
            out[seq, E + 128:] = qt[:, 1, j]
    return out
